# revision 2
# baseline (speedup 1.0000x reference)
"""Block-sparse causal self-attention on 8 TRN2 NeuronCores (SPMD Bass/Tile).

Sharding: core c -> (batch b = c//2, head-group g = c%2 of 6 heads).
Each core computes the qkv projection for its 6 heads, masked attention,
and a partial output projection (its 384 rows of W_proj); the host sums
the two partials per batch.

Token order (host permutation, inverted on output):
  [U_0 .. U_7 | A],  U_i = [tactile_i (16), image_i (196)]  (212 tokens),
  A = 9 action tokens.  In this order visibility is exactly
  "key-frame <= query-frame" with frame(U_i)=i, frame(action query r)=r,
  frame(action key j)=j-1.

Masking is folded into the score matmul as 7 extra contraction rows
(one per frame boundary b=1..7): row b of K^T holds -15*[fk(k)>=b], row b
of Q^T holds 16*[fq(q)<b]; each crossed boundary adds -240 to the score,
so exp(score/8) underflows to 0.  No mask tensor, no elementwise masking.

All matmul operands are float16 (fp32 PSUM accumulation); softmax
normalization comes from a ones-column appended to V (the rowsum d lands
at psum partition 64), a DVE reciprocal of that row, and a 1-row f32r
matmul that broadcasts 1/d over the 64 output partitions.
"""

import os
import sys
from contextlib import ExitStack

import numpy as np

for _p in ("/opt/trn_rl_repo", "/root/.axon_site/_ro/trn_rl_repo"):
    if os.path.isdir(_p) and _p not in sys.path:
        sys.path.insert(0, _p)

import concourse.bass as bass
import concourse.tile as tile
from concourse import mybir
from concourse.bass_utils import run_bass_kernel_spmd

F32 = mybir.dt.float32
F16 = mybir.dt.float16
AF = mybir.ActivationFunctionType

L, PP, PT = 8, 196, 16
T, C, NH, B, HD = 1705, 768, 12, 4, 64
NCORES = 8
NHG = NH // 2          # heads per core = 6
NPACK = NHG // 2       # head pairs per core = 3
KC = C // 128          # 6 contraction tiles over C
KT = 128
NKT = (T + KT - 1) // KT   # 14 key tiles; tile 13 has 41 keys
NB = 7                 # mask boundary rows
CR = HD + NB           # S contraction rows = 71
TPP = 1708             # T padded to a multiple of 4
FR = 212               # tokens per frame-block U_i
QCH = [(0, 424), (424, 848), (848, 1272), (1272, 1708)]
NCC = [424, 424, 424, 436]
MT = [4, 7, 10, 14]    # U key-tiles visible per query chunk
JQ, JK = 16.0, -15.0   # boundary row scales; product = -240 per crossing


def _perm():
    idx = []
    for i in range(L):
        idx += list(range(9 + PT * i, 9 + PT * (i + 1)))
        idx += list(range(9 + L * PT + PP * i, 9 + L * PT + PP * (i + 1)))
    idx += list(range(0, 9))
    return np.asarray(idx, dtype=np.int64)


PERM = _perm()


def _frames():
    fq = np.zeros(TPP, np.int64)
    fk = np.zeros(TPP, np.int64)
    for t in range(T):
        if t < 1696:
            fq[t] = t // FR
            fk[t] = t // FR
        else:
            fq[t] = t - 1696
            fk[t] = t - 1696 - 1
    # pad queries (1705..1707): fq stays 0 is fine only with zero qmask rows;
    # handled by writing zeros there explicitly in _mask_rows.
    return fq, fk


def _mask_rows():
    fq, fk = _frames()
    qm = np.zeros((NB, TPP), np.float32)
    km = np.zeros((NB, TPP), np.float32)
    for b in range(1, NB + 1):
        qm[b - 1, :T] = JQ * (fq[:T] < b)
        km[b - 1, :T] = JK * (fk[:T] >= b)
    return qm.astype(np.float16), km.astype(np.float16)


def _plan():
    """Per chunk: list of (tile, kw, qlo)."""
    plan = []
    for c in range(4):
        n = NCC[c]
        items = []
        for t in range(MT[c]):
            kw = min(KT, T - t * KT)
            fkmin = (t * KT) // FR
            qlo = max(0, fkmin - 2 * c) * FR
            qlo = min(qlo, n - 4)
            if t == NKT - 1:
                qlo = 0  # last tile holds the action keys, visible to all
            items.append((t, kw, qlo))
        if MT[c] < NKT:
            items.append((NKT - 1, T - (NKT - 1) * KT, 0))
        plan.append(items)
    return plan


PLAN = _plan()
# token tiles whose columns are complete after chunk c
PROJ_TILES = [[0, 1, 2], [3, 4, 5], [6, 7, 8], [9, 10, 11, 12, 13]]


def _split_excess_waits(nc, max_waits=1):
    """walrus rejects instructions with >2 sem-wait commands; split the
    excess onto preceding same-engine NoOps/Drains."""
    import copy

    for bb in nc.main_func.blocks:
        insts = bb.instructions
        i = 0
        while i < len(insts):
            ins = insts[i]
            si = ins.sync_info
            mw = max_waits
            if si is not None and len(si.on_wait) > mw:
                waits = list(si.on_wait)
                extra = waits[:-mw]
                newones = []
                for j in range(0, len(extra), max_waits):
                    if ins.__class__.__name__ == "InstDrain":
                        d = mybir.InstDrain(name=f"{ins.name}-sw{j}", engine=ins.engine)
                    else:
                        d = mybir.InstNoOp(name=f"{ins.name}-sw{j}", engine=ins.engine)
                    si2 = copy.deepcopy(si)
                    si2.on_wait = extra[j:j + max_waits]
                    si2.on_update = []
                    d.sync_info = si2
                    newones.append(d)
                si.on_wait = waits[-mw:]
                for d in reversed(newones):
                    insts.insert(i, d)
                i += len(newones)
            i += 1


_BUILD_CACHE = {}


def _build():
    if "nc" in _BUILD_CACHE:
        return _BUILD_CACHE["nc"]

    nc = bass.Bass()
    xT = nc.declare_dram_parameter("xT", [C, TPP], F16, isOutput=False)
    wa = nc.declare_dram_parameter("wa", [C, 3 * NHG * HD], F16, isOutput=False)
    wp = nc.declare_dram_parameter("wp", [NHG * HD, C], F16, isOutput=False)
    qm = nc.declare_dram_parameter("qm", [NB, TPP], F16, isOutput=False)
    km = nc.declare_dram_parameter("km", [NB, TPP], F16, isOutput=False)
    out = nc.declare_dram_parameter("out", [T, C], F32, isOutput=True)

    with tile.TileContext(nc) as tc:
        with ExitStack() as ctx:
            const = ctx.enter_context(tc.tile_pool(name="const", bufs=1))
            etp = ctx.enter_context(tc.tile_pool(name="etp", bufs=8))
            lnp = ctx.enter_context(tc.tile_pool(name="lnp", bufs=2))
            rcp = ctx.enter_context(tc.tile_pool(name="rcp", bufs=2))
            osb = ctx.enter_context(tc.tile_pool(name="osb", bufs=4))
            sps = ctx.enter_context(tc.tile_pool(name="sps", bufs=2, space="PSUM"))
            ups = ctx.enter_context(tc.tile_pool(name="ups", bufs=2, space="PSUM"))
            mmp = ctx.enter_context(tc.tile_pool(name="mmp", bufs=2, space="PSUM"))

            xt_sb = [const.tile([128, TPP], F16, tag=f"xt{k}", name=f"xt{k}")
                     for k in range(KC)]
            wa_sb = [const.tile([128, 3 * NHG * HD], F16, tag=f"wa{k}",
                                name=f"wa{k}") for k in range(KC)]
            # interleave x / W tiles within both queues so the first V and
            # QK accumulation steps can start as soon as possible
            for k in range(KC):
                enga = nc.scalar if k % 2 == 0 else nc.sync
                engx = nc.sync if k % 2 == 0 else nc.scalar
                enga.dma_start(out=wa_sb[k][:, :], in_=wa[k * 128:(k + 1) * 128, :])
                engx.dma_start(out=xt_sb[k][:, :], in_=xT[k * 128:(k + 1) * 128, :])
            wp_sb = []
            for p in range(NPACK):
                t_ = const.tile([128, C], F16, tag=f"wp{p}", name=f"wp{p}")
                nc.scalar.dma_start(out=t_[:, :], in_=wp[p * 128:(p + 1) * 128, :])
                wp_sb.append(t_)

            qt_sb = []
            kt_sb = []
            for h in range(NHG):
                tq = const.tile([72, TPP], F16, tag=f"qt{h}", name=f"qt{h}")
                tk = const.tile([72, TPP], F16, tag=f"kt{h}", name=f"ktt{h}")
                nc.scalar.dma_start(out=tq[HD:HD + NB, :], in_=qm[:, :])
                nc.scalar.dma_start(out=tk[HD:HD + NB, :], in_=km[:, :])
                qt_sb.append(tq)
                kt_sb.append(tk)

            v6_sb = []
            for t in range(NKT):
                t_ = const.tile([128, NHG, HD + 2], F16, tag=f"v6{t}", name=f"v6{t}")
                nc.gpsimd.memset(t_[:, :, HD:HD + 2], 0.0)
                v6_sb.append(t_)

            yt_sb = [const.tile([128, TPP], F16, tag=f"yt{p}", name=f"yt{p}")
                     for p in range(NPACK)]

            F32R = mybir.dt.float32r
            ones64 = const.tile([65, 64], F32R, tag="ones64", name="ones64")
            nc.vector.memset(ones64[HD:HD + 1, :].bitcast(F32), 1.0)

            # ---------------- work-piece emitters ----------------
            def emit_v(t):
                tw = min(KT, T - t * KT)
                ps = mmp.tile([128, 512], F32, tag="mm", name="vps")
                for k in range(KC):
                    nc.tensor.matmul(
                        ps[0:tw, 0:NHG * HD],
                        xt_sb[k][:, t * KT:t * KT + tw],
                        wa_sb[k][:, 2 * NHG * HD:3 * NHG * HD],
                        start=(k == 0), stop=(k == KC - 1),
                    )
                psv = ps[:, 0:NHG * HD].rearrange("a (h d) -> a h d", d=HD)
                nc.vector.tensor_copy(v6_sb[t][0:tw, :, 0:HD], psv[0:tw, :, :])
                nc.gpsimd.memset(v6_sb[t][0:tw, :, HD:HD + 1], 1.0)

            def emit_qk(c, p, j):
                q0, n = QCH[c][0], NCC[c]
                dst = qt_sb if j == 0 else kt_sb
                ps = mmp.tile([128, 512], F32, tag="mm", name="qkps")
                col = j * NHG * HD + p * 128
                for k in range(KC):
                    nc.tensor.matmul(
                        ps[:, 0:n],
                        wa_sb[k][:, col:col + 128],
                        xt_sb[k][:, q0:q0 + n],
                        start=(k == 0), stop=(k == KC - 1),
                    )
                nc.vector.tensor_copy(dst[2 * p][0:HD, q0:q0 + n], ps[0:HD, 0:n])
                nc.vector.tensor_copy(dst[2 * p + 1][0:HD, q0:q0 + n], ps[HD:128, 0:n])

            def emit_proj(t, half):
                tw = min(KT, T - t * KT)
                po = mmp.tile([128, 512], F32, tag="mm", name="pops")
                for p in range(NPACK):
                    nc.tensor.matmul(
                        po[0:tw, 0:384],
                        yt_sb[p][:, t * KT:t * KT + tw],
                        wp_sb[p][:, half * 384:half * 384 + 384],
                        start=(p == 0), stop=(p == NPACK - 1),
                    )
                ot = osb.tile([128, 384], F32, tag="ot", name="ot_sb")
                nc.vector.tensor_copy(ot[0:tw, :], po[0:tw, 0:384])
                nc.sync.dma_start(
                    out=out[t * KT:t * KT + tw, half * 384:half * 384 + 384],
                    in_=ot[0:tw, :],
                )

            fillers = []
            done = set()

            def fill(k=1):
                for _ in range(k):
                    if fillers:
                        tag, fn = fillers.pop(0)
                        fn()
                        done.add(tag)

            def need(tag):
                while fillers and tag not in done:
                    fill()

            # ---------------- startup: just enough for (c0, p0) ----------------
            emit_qk(0, 0, 0)
            emit_qk(0, 0, 1)
            done.add(("qk", 0, 0))
            v_first = [0, 1]
            for t in v_first:
                emit_v(t)
                done.add(("v", t))

            fillers.extend(
                (("v", t), (lambda t=t: emit_v(t)))
                for (t, kw, qlo) in PLAN[0] if t not in v_first
            )
            for p in (1, 2):
                fillers.append((("qk", 0, p),
                                (lambda p=p: (emit_qk(0, p, 0), emit_qk(0, p, 1)))))
            # all remaining QK production queued up-front: attention on key
            # tile t reads K^T columns produced by every chunk overlapping
            # that tile, which need() resolves before the S matmul
            for cq in (1, 2, 3):
                for p in range(NPACK):
                    fillers.append((("qk", cq, p),
                                    (lambda cq=cq, p=p: (emit_qk(cq, p, 0),
                                                         emit_qk(cq, p, 1)))))
            fillers.extend(
                (("v", t), (lambda t=t: emit_v(t)))
                for t in range(NKT)
                if t not in v_first and ("v", t) not in [f[0] for f in fillers]
            )

            def kt_chunks(t, kw):
                k0, k1 = t * KT, t * KT + kw
                return [cc for cc in range(4)
                        if QCH[cc][0] < k1 and k0 < QCH[cc][0] + NCC[cc]]

            # ---------------- per-chunk pipeline ----------------
            for c in range(4):
                q0 = QCH[c][0]
                n = NCC[c]
                if c >= 1:
                    fillers.extend(
                        ((("proj", t, half)),
                         (lambda t=t, half=half: emit_proj(t, half)))
                        for t in PROJ_TILES[c - 1] for half in (0, 1)
                    )

                # attention per pack
                for p in range(NPACK):
                    items = PLAN[c]
                    need(("qk", c, p))
                    u2 = [ups.tile([66, 512], F32, tag="u", name=f"ut{e}")
                          for e in (0, 1)]
                    pend = []
                    for idx, (t, kw, qlo) in enumerate(items):
                        need(("v", t))
                        for ck in kt_chunks(t, kw):
                            need(("qk", ck, p))
                        st = sps.tile([128, 2, 512], F32, tag="s", name="st")
                        for e in (0, 1):
                            nc.tensor.matmul(
                                st[0:kw, e, qlo:n],
                                kt_sb[2 * p + e][0:CR, t * KT:t * KT + kw],
                                qt_sb[2 * p + e][0:CR, q0 + qlo:q0 + n],
                                start=True, stop=True,
                            )
                        et = etp.tile([128, 2, 448], F16, tag="e", name="et")
                        nc.scalar.activation(
                            et[0:kw, :, qlo:n], st[0:kw, :, qlo:n], AF.Exp, scale=0.125
                        )
                        pend.append((idx, t, kw, qlo, et))
                        fill()
                        if len(pend) > 4:
                            _pv(nc, u2, v6_sb, p, n, pend.pop(0), len(items))
                    while pend:
                        _pv(nc, u2, v6_sb, p, n, pend.pop(0), len(items))

                    # normalization: 1/d on DVE, matmul-broadcast over the
                    # 64 hd partitions, then a psum*sbuf multiply into yt
                    fill()
                    ra = lnp.tile([65, 2, 512], mybir.dt.float32r,
                                  tag="ln", name="ra")
                    with nc.allow_low_precision("1/d broadcast via fp32r matmul"):
                        for e in (0, 1):
                            nc.vector.reciprocal(
                                ra[HD:HD + 1, e, 0:n], u2[e][HD:HD + 1, 0:n]
                            )
                    rbs = rcp.tile([64, 2, 512], F16, tag="rc", name="rbs")
                    for e in (0, 1):
                        rb = mmp.tile([128, 512], F32, tag="mm", name="rbps")
                        nc.tensor.matmul(
                            rb[0:64, 0:n],
                            ones64[HD:HD + 1, 0:64],
                            ra[HD:HD + 1, e, 0:n],
                            start=True, stop=True,
                        )
                        nc.vector.tensor_copy(rbs[0:64, e, 0:n], rb[0:64, 0:n])
                        nc.vector.tensor_mul(
                            yt_sb[p][e * 64:(e + 1) * 64, q0:q0 + n],
                            u2[e][0:64, 0:n],
                            rbs[0:64, e, 0:n],
                        )

                # drain remaining fillers before the next chunk's attention
                while fillers:
                    fill()

            # final output projection tiles
            for t in PROJ_TILES[3]:
                for half in (0, 1):
                    emit_proj(t, half)

    _split_excess_waits(nc)
    _BUILD_CACHE["nc"] = nc
    return nc


def _pv(nc, u2, v6_sb, p, n, item, nitems):
    idx, t, kw, qlo, et = item
    for e in (0, 1):
        nc.tensor.matmul(
            u2[e][0:66, qlo:n],
            v6_sb[t][0:kw, 2 * p + e, :],
            et[0:kw, e, qlo:n],
            start=(idx == 0), stop=(idx == nitems - 1),
        )


def _prep_inputs(x, W_attn, W_proj):
    x = np.asarray(x, np.float32)
    W_attn = np.asarray(W_attn, np.float32)
    W_proj = np.asarray(W_proj, np.float32)
    qmr, kmr = _mask_rows()
    xT_by_batch = []
    for b in range(B):
        xt = np.zeros((C, TPP), np.float16)
        xt[:, :T] = x[b][PERM, :].T
        xT_by_batch.append(xt)
    in_maps = []
    for core in range(NCORES):
        b, g = core // 2, core % 2
        qb, kb, vb = [], [], []
        for h in range(NHG):
            gh = g * NHG + h
            qb.append(W_attn[:, gh * HD:(gh + 1) * HD])
            kb.append(W_attn[:, C + gh * HD:C + (gh + 1) * HD])
            vb.append(W_attn[:, 2 * C + gh * HD:2 * C + (gh + 1) * HD])
        wa_core = np.concatenate(qb + kb + vb, axis=1).astype(np.float16)
        wp_core = np.ascontiguousarray(
            W_proj[g * NHG * HD:(g + 1) * NHG * HD, :]
        ).astype(np.float16)
        in_maps.append({
            "xT": xT_by_batch[b],
            "wa": np.ascontiguousarray(wa_core),
            "wp": wp_core,
            "qm": qmr,
            "km": kmr,
        })
    return in_maps


def _run(inputs, trace=False, trace_cores=None):
    nc = _build()
    in_maps = _prep_inputs(inputs["x"], inputs["W_attn"], inputs["W_proj"])
    res = run_bass_kernel_spmd(
        nc, in_maps, list(range(NCORES)), trace=trace, trace_cores=trace_cores
    )
    outs = [np.asarray(r["out"], np.float32) for r in res.results]
    full = np.empty((B, T, C), np.float32)
    for b in range(B):
        comb = outs[2 * b] + outs[2 * b + 1]
        full[b][PERM, :] = comb
    return full, res


def kernel(**inputs) -> np.ndarray:
    out, _ = _run(inputs)
    return out


# revision 3
# speedup vs baseline: 1.0075x; 1.0075x over previous
"""Block-sparse causal self-attention on 8 TRN2 NeuronCores (SPMD Bass/Tile).

Sharding: core c -> (batch b = c//2, head-group g = c%2 of 6 heads).
Each core computes the qkv projection for its 6 heads, masked attention,
and a partial output projection (its 384 rows of W_proj); the host sums
the two partials per batch.

Token order (host permutation, inverted on output):
  [U_0 .. U_7 | A],  U_i = [tactile_i (16), image_i (196)]  (212 tokens),
  A = 9 action tokens.  In this order visibility is exactly
  "key-frame <= query-frame" with frame(U_i)=i, frame(action query r)=r,
  frame(action key j)=j-1.

Masking is folded into the score matmul as 7 extra contraction rows
(one per frame boundary b=1..7): row b of K^T holds -15*[fk(k)>=b], row b
of Q^T holds 16*[fq(q)<b]; each crossed boundary adds -240 to the score,
so exp(score/8) underflows to 0.  No mask tensor, no elementwise masking.

All matmul operands are float16 (fp32 PSUM accumulation); softmax
normalization comes from a ones-column appended to V (the rowsum d lands
at psum partition 64), a DVE reciprocal of that row, and a 1-row f32r
matmul that broadcasts 1/d over the 64 output partitions.
"""

import os
import sys
from contextlib import ExitStack

import numpy as np

for _p in ("/opt/trn_rl_repo", "/root/.axon_site/_ro/trn_rl_repo"):
    if os.path.isdir(_p) and _p not in sys.path:
        sys.path.insert(0, _p)

import concourse.bass as bass
import concourse.tile as tile
from concourse import mybir
from concourse.bass_utils import run_bass_kernel_spmd

F32 = mybir.dt.float32
F16 = mybir.dt.float16
AF = mybir.ActivationFunctionType

L, PP, PT = 8, 196, 16
T, C, NH, B, HD = 1705, 768, 12, 4, 64
NCORES = 8
NHG = NH // 2          # heads per core = 6
NPACK = NHG // 2       # head pairs per core = 3
KC = C // 128          # 6 contraction tiles over C
KT = 128
NKT = (T + KT - 1) // KT   # 14 key tiles; tile 13 has 41 keys
NB = 7                 # mask boundary rows
CR = HD + NB           # S contraction rows = 71
TPP = 1708             # T padded to a multiple of 4
FR = 212               # tokens per frame-block U_i
QCH = [(0, 424), (424, 848), (848, 1272), (1272, 1708)]
NCC = [424, 424, 424, 436]
MT = [4, 7, 10, 14]    # U key-tiles visible per query chunk
JQ, JK = 16.0, -15.0   # boundary row scales; product = -240 per crossing


def _perm():
    idx = []
    for i in range(L):
        idx += list(range(9 + PT * i, 9 + PT * (i + 1)))
        idx += list(range(9 + L * PT + PP * i, 9 + L * PT + PP * (i + 1)))
    idx += list(range(0, 9))
    return np.asarray(idx, dtype=np.int64)


PERM = _perm()


def _frames():
    fq = np.zeros(TPP, np.int64)
    fk = np.zeros(TPP, np.int64)
    for t in range(T):
        if t < 1696:
            fq[t] = t // FR
            fk[t] = t // FR
        else:
            fq[t] = t - 1696
            fk[t] = t - 1696 - 1
    # pad queries (1705..1707): fq stays 0 is fine only with zero qmask rows;
    # handled by writing zeros there explicitly in _mask_rows.
    return fq, fk


def _mask_rows():
    fq, fk = _frames()
    qm = np.zeros((NB, TPP), np.float32)
    km = np.zeros((NB, TPP), np.float32)
    for b in range(1, NB + 1):
        qm[b - 1, :T] = JQ * (fq[:T] < b)
        km[b - 1, :T] = JK * (fk[:T] >= b)
    return qm.astype(np.float16), km.astype(np.float16)


def _plan():
    """Per chunk: list of (tile, kw, qlo)."""
    plan = []
    for c in range(4):
        n = NCC[c]
        items = []
        for t in range(MT[c]):
            kw = min(KT, T - t * KT)
            fkmin = (t * KT) // FR
            qlo = max(0, fkmin - 2 * c) * FR
            qlo = min(qlo, n - 4)
            if t == NKT - 1:
                qlo = 0  # last tile holds the action keys, visible to all
            items.append((t, kw, qlo))
        if MT[c] < NKT:
            items.append((NKT - 1, T - (NKT - 1) * KT, 0))
        plan.append(items)
    return plan


PLAN = _plan()
# token tiles whose columns are complete after chunk c
PROJ_TILES = [[0, 1, 2], [3, 4, 5], [6, 7, 8], [9, 10, 11, 12, 13]]


def _split_excess_waits(nc, max_waits=1):
    """walrus rejects instructions with >2 sem-wait commands; split the
    excess onto preceding same-engine NoOps/Drains."""
    import copy

    for bb in nc.main_func.blocks:
        insts = bb.instructions
        i = 0
        while i < len(insts):
            ins = insts[i]
            si = ins.sync_info
            mw = max_waits
            if si is not None and len(si.on_wait) > mw:
                waits = list(si.on_wait)
                extra = waits[:-mw]
                newones = []
                for j in range(0, len(extra), max_waits):
                    if ins.__class__.__name__ == "InstDrain":
                        d = mybir.InstDrain(name=f"{ins.name}-sw{j}", engine=ins.engine)
                    else:
                        d = mybir.InstNoOp(name=f"{ins.name}-sw{j}", engine=ins.engine)
                    si2 = copy.deepcopy(si)
                    si2.on_wait = extra[j:j + max_waits]
                    si2.on_update = []
                    d.sync_info = si2
                    newones.append(d)
                si.on_wait = waits[-mw:]
                for d in reversed(newones):
                    insts.insert(i, d)
                i += len(newones)
            i += 1


_BUILD_CACHE = {}


def _build():
    if "nc" in _BUILD_CACHE:
        return _BUILD_CACHE["nc"]

    nc = bass.Bass()
    xT = nc.declare_dram_parameter("xT", [C, TPP], F16, isOutput=False)
    wa = nc.declare_dram_parameter("wa", [C, 3 * NHG * HD], F16, isOutput=False)
    wp = nc.declare_dram_parameter("wp", [NHG * HD, C], F16, isOutput=False)
    qm = nc.declare_dram_parameter("qm", [NB, TPP], F16, isOutput=False)
    km = nc.declare_dram_parameter("km", [NB, TPP], F16, isOutput=False)
    out = nc.declare_dram_parameter("out", [T, C], F32, isOutput=True)

    with tile.TileContext(nc) as tc:
        with ExitStack() as ctx:
            const = ctx.enter_context(tc.tile_pool(name="const", bufs=1))
            etp = ctx.enter_context(tc.tile_pool(name="etp", bufs=8))
            lnp = ctx.enter_context(tc.tile_pool(name="lnp", bufs=3))
            rcp = ctx.enter_context(tc.tile_pool(name="rcp", bufs=3))
            osb = ctx.enter_context(tc.tile_pool(name="osb", bufs=6))
            sps = ctx.enter_context(tc.tile_pool(name="sps", bufs=2, space="PSUM"))
            ups = ctx.enter_context(tc.tile_pool(name="ups", bufs=2, space="PSUM"))
            mmp = ctx.enter_context(tc.tile_pool(name="mmp", bufs=2, space="PSUM"))

            xt_sb = [const.tile([128, TPP], F16, tag=f"xt{k}", name=f"xt{k}")
                     for k in range(KC)]
            wa_sb = [const.tile([128, 3 * NHG * HD], F16, tag=f"wa{k}",
                                name=f"wa{k}") for k in range(KC)]
            # interleave x / W tiles within both queues so the first V and
            # QK accumulation steps can start as soon as possible
            for k in range(KC):
                enga = nc.scalar if k % 2 == 0 else nc.sync
                engx = nc.sync if k % 2 == 0 else nc.scalar
                enga.dma_start(out=wa_sb[k][:, :], in_=wa[k * 128:(k + 1) * 128, :])
                engx.dma_start(out=xt_sb[k][:, :], in_=xT[k * 128:(k + 1) * 128, :])
            wp_sb = []
            for p in range(NPACK):
                t_ = const.tile([128, C], F16, tag=f"wp{p}", name=f"wp{p}")
                nc.scalar.dma_start(out=t_[:, :], in_=wp[p * 128:(p + 1) * 128, :])
                wp_sb.append(t_)

            qt_sb = []
            kt_sb = []
            for h in range(NHG):
                tq = const.tile([72, TPP], F16, tag=f"qt{h}", name=f"qt{h}")
                tk = const.tile([72, TPP], F16, tag=f"kt{h}", name=f"ktt{h}")
                nc.scalar.dma_start(out=tq[HD:HD + NB, :], in_=qm[:, :])
                nc.scalar.dma_start(out=tk[HD:HD + NB, :], in_=km[:, :])
                qt_sb.append(tq)
                kt_sb.append(tk)

            v6_sb = []
            for t in range(NKT):
                t_ = const.tile([128, NHG, HD + 2], F16, tag=f"v6{t}", name=f"v6{t}")
                nc.gpsimd.memset(t_[:, :, HD:HD + 2], 0.0)
                v6_sb.append(t_)

            yt_sb = [const.tile([128, TPP], F16, tag=f"yt{p}", name=f"yt{p}")
                     for p in range(NPACK)]

            F32R = mybir.dt.float32r
            ones64 = const.tile([65, 64], F32R, tag="ones64", name="ones64")
            nc.vector.memset(ones64[HD:HD + 1, :].bitcast(F32), 1.0)

            # ---------------- work-piece emitters ----------------
            def emit_v(t):
                tw = min(KT, T - t * KT)
                ps = mmp.tile([128, 512], F32, tag="mm", name="vps")
                for k in range(KC):
                    nc.tensor.matmul(
                        ps[0:tw, 0:NHG * HD],
                        xt_sb[k][:, t * KT:t * KT + tw],
                        wa_sb[k][:, 2 * NHG * HD:3 * NHG * HD],
                        start=(k == 0), stop=(k == KC - 1),
                    )
                psv = ps[:, 0:NHG * HD].rearrange("a (h d) -> a h d", d=HD)
                nc.vector.tensor_copy(v6_sb[t][0:tw, :, 0:HD], psv[0:tw, :, :])
                nc.gpsimd.memset(v6_sb[t][0:tw, :, HD:HD + 1], 1.0)

            def emit_qk(c, p, j):
                q0, n = QCH[c][0], NCC[c]
                dst = qt_sb if j == 0 else kt_sb
                ps = mmp.tile([128, 512], F32, tag="mm", name="qkps")
                col = j * NHG * HD + p * 128
                for k in range(KC):
                    nc.tensor.matmul(
                        ps[:, 0:n],
                        wa_sb[k][:, col:col + 128],
                        xt_sb[k][:, q0:q0 + n],
                        start=(k == 0), stop=(k == KC - 1),
                    )
                nc.vector.tensor_copy(dst[2 * p][0:HD, q0:q0 + n], ps[0:HD, 0:n])
                nc.vector.tensor_copy(dst[2 * p + 1][0:HD, q0:q0 + n], ps[HD:128, 0:n])

            def emit_proj(t, half):
                tw = min(KT, T - t * KT)
                po = mmp.tile([128, 512], F32, tag="mm", name="pops")
                for p in range(NPACK):
                    nc.tensor.matmul(
                        po[0:tw, 0:384],
                        yt_sb[p][:, t * KT:t * KT + tw],
                        wp_sb[p][:, half * 384:half * 384 + 384],
                        start=(p == 0), stop=(p == NPACK - 1),
                    )
                ot = osb.tile([128, 384], F32, tag="ot", name="ot_sb")
                nc.vector.tensor_copy(ot[0:tw, :], po[0:tw, 0:384])
                nc.sync.dma_start(
                    out=out[t * KT:t * KT + tw, half * 384:half * 384 + 384],
                    in_=ot[0:tw, :],
                )

            fillers = []
            done = set()

            def fill(k=1):
                for _ in range(k):
                    if fillers:
                        tag, fn = fillers.pop(0)
                        fn()
                        done.add(tag)

            def need(tag):
                while fillers and tag not in done:
                    fill()

            # ---------------- startup: just enough for (c0, p0) ----------------
            emit_qk(0, 0, 0)
            emit_qk(0, 0, 1)
            done.add(("qk", 0, 0))
            v_first = [0, 1]
            for t in v_first:
                emit_v(t)
                done.add(("v", t))

            fillers.extend(
                (("v", t), (lambda t=t: emit_v(t)))
                for (t, kw, qlo) in PLAN[0] if t not in v_first
            )
            for p in (1, 2):
                fillers.append((("qk", 0, p),
                                (lambda p=p: (emit_qk(0, p, 0), emit_qk(0, p, 1)))))
            # all remaining QK production queued up-front: attention on key
            # tile t reads K^T columns produced by every chunk overlapping
            # that tile, which need() resolves before the S matmul
            for cq in (1, 2, 3):
                for p in range(NPACK):
                    fillers.append((("qk", cq, p),
                                    (lambda cq=cq, p=p: (emit_qk(cq, p, 0),
                                                         emit_qk(cq, p, 1)))))
            fillers.extend(
                (("v", t), (lambda t=t: emit_v(t)))
                for t in range(NKT)
                if t not in v_first and ("v", t) not in [f[0] for f in fillers]
            )

            def kt_chunks(t, kw):
                k0, k1 = t * KT, t * KT + kw
                return [cc for cc in range(4)
                        if QCH[cc][0] < k1 and k0 < QCH[cc][0] + NCC[cc]]

            # ---------------- per-chunk pipeline ----------------
            for c in range(4):
                q0 = QCH[c][0]
                n = NCC[c]
                if c >= 1:
                    fillers.extend(
                        ((("proj", t, half)),
                         (lambda t=t, half=half: emit_proj(t, half)))
                        for t in PROJ_TILES[c - 1] for half in (0, 1)
                    )

                # attention per pack
                for p in range(NPACK):
                    items = PLAN[c]
                    need(("qk", c, p))
                    u2 = [ups.tile([66, 512], F32, tag="u", name=f"ut{e}")
                          for e in (0, 1)]
                    pend = []
                    for idx, (t, kw, qlo) in enumerate(items):
                        need(("v", t))
                        for ck in kt_chunks(t, kw):
                            need(("qk", ck, p))
                        st = sps.tile([128, 2, 512], F32, tag="s", name="st")
                        for e in (0, 1):
                            nc.tensor.matmul(
                                st[0:kw, e, qlo:n],
                                kt_sb[2 * p + e][0:CR, t * KT:t * KT + kw],
                                qt_sb[2 * p + e][0:CR, q0 + qlo:q0 + n],
                                start=True, stop=True,
                            )
                        et = etp.tile([128, 2, 448], F16, tag="e", name="et")
                        nc.scalar.activation(
                            et[0:kw, :, qlo:n], st[0:kw, :, qlo:n], AF.Exp, scale=0.125
                        )
                        pend.append((idx, t, kw, qlo, et))
                        fill()
                        if len(pend) > 4:
                            _pv(nc, u2, v6_sb, p, n, pend.pop(0), len(items))
                    while pend:
                        _pv(nc, u2, v6_sb, p, n, pend.pop(0), len(items))

                    # normalization: 1/d on DVE, matmul-broadcast over the
                    # 64 hd partitions, then a psum*sbuf multiply into yt
                    fill()
                    ra = lnp.tile([65, 2, 512], mybir.dt.float32r,
                                  tag="ln", name="ra")
                    with nc.allow_low_precision("1/d broadcast via fp32r matmul"):
                        for e in (0, 1):
                            nc.vector.reciprocal(
                                ra[HD:HD + 1, e, 0:n], u2[e][HD:HD + 1, 0:n]
                            )
                    rbs = rcp.tile([64, 2, 512], F16, tag="rc", name="rbs")
                    for e in (0, 1):
                        rb = mmp.tile([128, 512], F32, tag="mm", name="rbps")
                        nc.tensor.matmul(
                            rb[0:64, 0:n],
                            ones64[HD:HD + 1, 0:64],
                            ra[HD:HD + 1, e, 0:n],
                            start=True, stop=True,
                        )
                        nc.vector.tensor_copy(rbs[0:64, e, 0:n], rb[0:64, 0:n])
                        nc.vector.tensor_mul(
                            yt_sb[p][e * 64:(e + 1) * 64, q0:q0 + n],
                            u2[e][0:64, 0:n],
                            rbs[0:64, e, 0:n],
                        )

                # drain remaining fillers before the next chunk's attention
                while fillers:
                    fill()

            # final output projection tiles
            for t in PROJ_TILES[3]:
                for half in (0, 1):
                    emit_proj(t, half)

    _split_excess_waits(nc)
    _BUILD_CACHE["nc"] = nc
    return nc


def _pv(nc, u2, v6_sb, p, n, item, nitems):
    idx, t, kw, qlo, et = item
    for e in (0, 1):
        nc.tensor.matmul(
            u2[e][0:66, qlo:n],
            v6_sb[t][0:kw, 2 * p + e, :],
            et[0:kw, e, qlo:n],
            start=(idx == 0), stop=(idx == nitems - 1),
        )


def _prep_inputs(x, W_attn, W_proj):
    x = np.asarray(x, np.float32)
    W_attn = np.asarray(W_attn, np.float32)
    W_proj = np.asarray(W_proj, np.float32)
    qmr, kmr = _mask_rows()
    xT_by_batch = []
    for b in range(B):
        xt = np.zeros((C, TPP), np.float16)
        xt[:, :T] = x[b][PERM, :].T
        xT_by_batch.append(xt)
    in_maps = []
    for core in range(NCORES):
        b, g = core // 2, core % 2
        qb, kb, vb = [], [], []
        for h in range(NHG):
            gh = g * NHG + h
            qb.append(W_attn[:, gh * HD:(gh + 1) * HD])
            kb.append(W_attn[:, C + gh * HD:C + (gh + 1) * HD])
            vb.append(W_attn[:, 2 * C + gh * HD:2 * C + (gh + 1) * HD])
        wa_core = np.concatenate(qb + kb + vb, axis=1).astype(np.float16)
        wp_core = np.ascontiguousarray(
            W_proj[g * NHG * HD:(g + 1) * NHG * HD, :]
        ).astype(np.float16)
        in_maps.append({
            "xT": xT_by_batch[b],
            "wa": np.ascontiguousarray(wa_core),
            "wp": wp_core,
            "qm": qmr,
            "km": kmr,
        })
    return in_maps


def _run(inputs, trace=False, trace_cores=None):
    nc = _build()
    in_maps = _prep_inputs(inputs["x"], inputs["W_attn"], inputs["W_proj"])
    res = run_bass_kernel_spmd(
        nc, in_maps, list(range(NCORES)), trace=trace, trace_cores=trace_cores
    )
    outs = [np.asarray(r["out"], np.float32) for r in res.results]
    full = np.empty((B, T, C), np.float32)
    for b in range(B):
        comb = outs[2 * b] + outs[2 * b + 1]
        full[b][PERM, :] = comb
    return full, res


def kernel(**inputs) -> np.ndarray:
    out, _ = _run(inputs)
    return out


# revision 4
# speedup vs baseline: 1.0080x; 1.0005x over previous
"""Block-sparse causal self-attention on 8 TRN2 NeuronCores (SPMD Bass/Tile).

Sharding: core c -> (batch b = c//2, head-group g = c%2 of 6 heads).
Each core computes the qkv projection for its 6 heads, masked attention,
and a partial output projection (its 384 rows of W_proj); the host sums
the two partials per batch.

Token order (host permutation, inverted on output):
  [U_0 .. U_7 | A],  U_i = [tactile_i (16), image_i (196)]  (212 tokens),
  A = 9 action tokens.  In this order visibility is exactly
  "key-frame <= query-frame" with frame(U_i)=i, frame(action query r)=r,
  frame(action key j)=j-1.

Masking is folded into the score matmul as 7 extra contraction rows
(one per frame boundary b=1..7): row b of K^T holds -15*[fk(k)>=b], row b
of Q^T holds 16*[fq(q)<b]; each crossed boundary adds -240 to the score,
so exp(score/8) underflows to 0.  No mask tensor, no elementwise masking.

All matmul operands are float16 (fp32 PSUM accumulation); softmax
normalization comes from a ones-column appended to V (the rowsum d lands
at psum partition 64), a DVE reciprocal of that row, and a 1-row f32r
matmul that broadcasts 1/d over the 64 output partitions.
"""

import os
import sys
from contextlib import ExitStack

import numpy as np

for _p in ("/opt/trn_rl_repo", "/root/.axon_site/_ro/trn_rl_repo"):
    if os.path.isdir(_p) and _p not in sys.path:
        sys.path.insert(0, _p)

import concourse.bass as bass
import concourse.tile as tile
from concourse import mybir
from concourse.bass_utils import run_bass_kernel_spmd

F32 = mybir.dt.float32
F16 = mybir.dt.float16
AF = mybir.ActivationFunctionType

L, PP, PT = 8, 196, 16
T, C, NH, B, HD = 1705, 768, 12, 4, 64
NCORES = 8
NHG = NH // 2          # heads per core = 6
NPACK = NHG // 2       # head pairs per core = 3
KC = C // 128          # 6 contraction tiles over C
KT = 128
NKT = (T + KT - 1) // KT   # 14 key tiles; tile 13 has 41 keys
NB = 7                 # mask boundary rows
CR = HD + NB           # S contraction rows = 71
TPP = 1708             # T padded to a multiple of 4
FR = 212               # tokens per frame-block U_i
QCH = [(0, 424), (424, 848), (848, 1272), (1272, 1708)]
NCC = [424, 424, 424, 436]
MT = [4, 7, 10, 14]    # U key-tiles visible per query chunk
JQ, JK = 16.0, -15.0   # boundary row scales; product = -240 per crossing


def _perm():
    idx = []
    for i in range(L):
        idx += list(range(9 + PT * i, 9 + PT * (i + 1)))
        idx += list(range(9 + L * PT + PP * i, 9 + L * PT + PP * (i + 1)))
    idx += list(range(0, 9))
    return np.asarray(idx, dtype=np.int64)


PERM = _perm()


def _frames():
    fq = np.zeros(TPP, np.int64)
    fk = np.zeros(TPP, np.int64)
    for t in range(T):
        if t < 1696:
            fq[t] = t // FR
            fk[t] = t // FR
        else:
            fq[t] = t - 1696
            fk[t] = t - 1696 - 1
    # pad queries (1705..1707): fq stays 0 is fine only with zero qmask rows;
    # handled by writing zeros there explicitly in _mask_rows.
    return fq, fk


def _mask_rows():
    fq, fk = _frames()
    qm = np.zeros((NB, TPP), np.float32)
    km = np.zeros((NB, TPP), np.float32)
    for b in range(1, NB + 1):
        qm[b - 1, :T] = JQ * (fq[:T] < b)
        km[b - 1, :T] = JK * (fk[:T] >= b)
    return qm.astype(np.float16), km.astype(np.float16)


def _plan():
    """Per chunk: list of (tile, kw, qlo)."""
    plan = []
    for c in range(4):
        n = NCC[c]
        items = []
        for t in range(MT[c]):
            kw = min(KT, T - t * KT)
            fkmin = (t * KT) // FR
            qlo = max(0, fkmin - 2 * c) * FR
            qlo = min(qlo, n - 4)
            if t == NKT - 1:
                qlo = 0  # last tile holds the action keys, visible to all
            items.append((t, kw, qlo))
        if MT[c] < NKT:
            items.append((NKT - 1, T - (NKT - 1) * KT, 0))
        plan.append(items)
    return plan


PLAN = _plan()
# token tiles whose columns are complete after chunk c
PROJ_TILES = [[0, 1, 2], [3, 4, 5], [6, 7, 8], [9, 10, 11, 12, 13]]


def _split_excess_waits(nc, max_waits=1):
    """walrus rejects instructions with >2 sem-wait commands; split the
    excess onto preceding same-engine NoOps/Drains."""
    import copy

    for bb in nc.main_func.blocks:
        insts = bb.instructions
        i = 0
        while i < len(insts):
            ins = insts[i]
            si = ins.sync_info
            mw = max_waits
            if si is not None and len(si.on_wait) > mw:
                waits = list(si.on_wait)
                extra = waits[:-mw]
                newones = []
                for j in range(0, len(extra), max_waits):
                    if ins.__class__.__name__ == "InstDrain":
                        d = mybir.InstDrain(name=f"{ins.name}-sw{j}", engine=ins.engine)
                    else:
                        d = mybir.InstNoOp(name=f"{ins.name}-sw{j}", engine=ins.engine)
                    si2 = copy.deepcopy(si)
                    si2.on_wait = extra[j:j + max_waits]
                    si2.on_update = []
                    d.sync_info = si2
                    newones.append(d)
                si.on_wait = waits[-mw:]
                for d in reversed(newones):
                    insts.insert(i, d)
                i += len(newones)
            i += 1


_BUILD_CACHE = {}


def _build():
    if "nc" in _BUILD_CACHE:
        return _BUILD_CACHE["nc"]

    nc = bass.Bass()
    xT = nc.declare_dram_parameter("xT", [C, TPP], F16, isOutput=False)
    wa = nc.declare_dram_parameter("wa", [C, 3 * NHG * HD], F16, isOutput=False)
    wp = nc.declare_dram_parameter("wp", [NHG * HD, C], F16, isOutput=False)
    qm = nc.declare_dram_parameter("qm", [NB, TPP], F16, isOutput=False)
    km = nc.declare_dram_parameter("km", [NB, TPP], F16, isOutput=False)
    out = nc.declare_dram_parameter("out", [T, C], F16, isOutput=True)

    with tile.TileContext(nc) as tc:
        with ExitStack() as ctx:
            const = ctx.enter_context(tc.tile_pool(name="const", bufs=1))
            etp = ctx.enter_context(tc.tile_pool(name="etp", bufs=8))
            lnp = ctx.enter_context(tc.tile_pool(name="lnp", bufs=3))
            rcp = ctx.enter_context(tc.tile_pool(name="rcp", bufs=3))
            osb = ctx.enter_context(tc.tile_pool(name="osb", bufs=6))
            sps = ctx.enter_context(tc.tile_pool(name="sps", bufs=2, space="PSUM"))
            ups = ctx.enter_context(tc.tile_pool(name="ups", bufs=2, space="PSUM"))
            mmp = ctx.enter_context(tc.tile_pool(name="mmp", bufs=2, space="PSUM"))

            xt_sb = [const.tile([128, TPP], F16, tag=f"xt{k}", name=f"xt{k}")
                     for k in range(KC)]
            wa_sb = [const.tile([128, 3 * NHG * HD], F16, tag=f"wa{k}",
                                name=f"wa{k}") for k in range(KC)]
            # interleave x / W tiles within both queues so the first V and
            # QK accumulation steps can start as soon as possible
            for k in range(KC):
                enga = nc.scalar if k % 2 == 0 else nc.sync
                engx = nc.sync if k % 2 == 0 else nc.scalar
                enga.dma_start(out=wa_sb[k][:, :], in_=wa[k * 128:(k + 1) * 128, :])
                engx.dma_start(out=xt_sb[k][:, :], in_=xT[k * 128:(k + 1) * 128, :])
            wp_sb = []
            for p in range(NPACK):
                t_ = const.tile([128, C], F16, tag=f"wp{p}", name=f"wp{p}")
                nc.scalar.dma_start(out=t_[:, :], in_=wp[p * 128:(p + 1) * 128, :])
                wp_sb.append(t_)

            qt_sb = []
            kt_sb = []
            for h in range(NHG):
                tq = const.tile([72, TPP], F16, tag=f"qt{h}", name=f"qt{h}")
                tk = const.tile([72, TPP], F16, tag=f"kt{h}", name=f"ktt{h}")
                nc.scalar.dma_start(out=tq[HD:HD + NB, :], in_=qm[:, :])
                nc.scalar.dma_start(out=tk[HD:HD + NB, :], in_=km[:, :])
                qt_sb.append(tq)
                kt_sb.append(tk)

            v6_sb = []
            for t in range(NKT):
                t_ = const.tile([128, NHG, HD + 2], F16, tag=f"v6{t}", name=f"v6{t}")
                nc.gpsimd.memset(t_[:, :, HD:HD + 2], 0.0)
                v6_sb.append(t_)

            yt_sb = [const.tile([128, TPP], F16, tag=f"yt{p}", name=f"yt{p}")
                     for p in range(NPACK)]

            F32R = mybir.dt.float32r
            ones64 = const.tile([65, 64], F32R, tag="ones64", name="ones64")
            nc.vector.memset(ones64[HD:HD + 1, :].bitcast(F32), 1.0)

            # ---------------- work-piece emitters ----------------
            def emit_v(t):
                tw = min(KT, T - t * KT)
                ps = mmp.tile([128, 512], F32, tag="mm", name="vps")
                for k in range(KC):
                    nc.tensor.matmul(
                        ps[0:tw, 0:NHG * HD],
                        xt_sb[k][:, t * KT:t * KT + tw],
                        wa_sb[k][:, 2 * NHG * HD:3 * NHG * HD],
                        start=(k == 0), stop=(k == KC - 1),
                    )
                psv = ps[:, 0:NHG * HD].rearrange("a (h d) -> a h d", d=HD)
                nc.vector.tensor_copy(v6_sb[t][0:tw, :, 0:HD], psv[0:tw, :, :])
                nc.gpsimd.memset(v6_sb[t][0:tw, :, HD:HD + 1], 1.0)

            def emit_qk(c, p, j):
                q0, n = QCH[c][0], NCC[c]
                dst = qt_sb if j == 0 else kt_sb
                ps = mmp.tile([128, 512], F32, tag="mm", name="qkps")
                col = j * NHG * HD + p * 128
                for k in range(KC):
                    nc.tensor.matmul(
                        ps[:, 0:n],
                        wa_sb[k][:, col:col + 128],
                        xt_sb[k][:, q0:q0 + n],
                        start=(k == 0), stop=(k == KC - 1),
                    )
                nc.vector.tensor_copy(dst[2 * p][0:HD, q0:q0 + n], ps[0:HD, 0:n])
                nc.vector.tensor_copy(dst[2 * p + 1][0:HD, q0:q0 + n], ps[HD:128, 0:n])

            def emit_proj(t, half):
                tw = min(KT, T - t * KT)
                po = mmp.tile([128, 512], F32, tag="mm", name="pops")
                for p in range(NPACK):
                    nc.tensor.matmul(
                        po[0:tw, 0:384],
                        yt_sb[p][:, t * KT:t * KT + tw],
                        wp_sb[p][:, half * 384:half * 384 + 384],
                        start=(p == 0), stop=(p == NPACK - 1),
                    )
                ot = osb.tile([128, 384], F16, tag="ot", name="ot_sb")
                nc.vector.tensor_copy(ot[0:tw, :], po[0:tw, 0:384])
                nc.sync.dma_start(
                    out=out[t * KT:t * KT + tw, half * 384:half * 384 + 384],
                    in_=ot[0:tw, :],
                )

            fillers = []
            done = set()

            def fill(k=1):
                for _ in range(k):
                    if fillers:
                        tag, fn = fillers.pop(0)
                        fn()
                        done.add(tag)

            def need(tag):
                while fillers and tag not in done:
                    fill()

            # ---------------- startup: just enough for (c0, p0) ----------------
            emit_qk(0, 0, 0)
            emit_qk(0, 0, 1)
            done.add(("qk", 0, 0))
            v_first = [0, 1]
            for t in v_first:
                emit_v(t)
                done.add(("v", t))

            fillers.extend(
                (("v", t), (lambda t=t: emit_v(t)))
                for (t, kw, qlo) in PLAN[0] if t not in v_first
            )
            for p in (1, 2):
                fillers.append((("qk", 0, p),
                                (lambda p=p: (emit_qk(0, p, 0), emit_qk(0, p, 1)))))
            # all remaining QK production queued up-front: attention on key
            # tile t reads K^T columns produced by every chunk overlapping
            # that tile, which need() resolves before the S matmul
            for cq in (1, 2, 3):
                for p in range(NPACK):
                    fillers.append((("qk", cq, p),
                                    (lambda cq=cq, p=p: (emit_qk(cq, p, 0),
                                                         emit_qk(cq, p, 1)))))
            fillers.extend(
                (("v", t), (lambda t=t: emit_v(t)))
                for t in range(NKT)
                if t not in v_first and ("v", t) not in [f[0] for f in fillers]
            )

            def kt_chunks(t, kw):
                k0, k1 = t * KT, t * KT + kw
                return [cc for cc in range(4)
                        if QCH[cc][0] < k1 and k0 < QCH[cc][0] + NCC[cc]]

            # ---------------- per-chunk pipeline ----------------
            for c in range(4):
                q0 = QCH[c][0]
                n = NCC[c]
                if c >= 1:
                    fillers.extend(
                        ((("proj", t, half)),
                         (lambda t=t, half=half: emit_proj(t, half)))
                        for t in PROJ_TILES[c - 1] for half in (0, 1)
                    )

                # attention per pack
                for p in range(NPACK):
                    items = PLAN[c]
                    need(("qk", c, p))
                    u2 = [ups.tile([66, 512], F32, tag="u", name=f"ut{e}")
                          for e in (0, 1)]
                    pend = []
                    for idx, (t, kw, qlo) in enumerate(items):
                        need(("v", t))
                        for ck in kt_chunks(t, kw):
                            need(("qk", ck, p))
                        st = sps.tile([128, 2, 512], F32, tag="s", name="st")
                        for e in (0, 1):
                            nc.tensor.matmul(
                                st[0:kw, e, qlo:n],
                                kt_sb[2 * p + e][0:CR, t * KT:t * KT + kw],
                                qt_sb[2 * p + e][0:CR, q0 + qlo:q0 + n],
                                start=True, stop=True,
                            )
                        et = etp.tile([128, 2, 448], F16, tag="e", name="et")
                        nc.scalar.activation(
                            et[0:kw, :, qlo:n], st[0:kw, :, qlo:n], AF.Exp, scale=0.125
                        )
                        pend.append((idx, t, kw, qlo, et))
                        fill()
                        if len(pend) > 4:
                            _pv(nc, u2, v6_sb, p, n, pend.pop(0), len(items))
                    while pend:
                        _pv(nc, u2, v6_sb, p, n, pend.pop(0), len(items))

                    # normalization: 1/d on DVE, matmul-broadcast over the
                    # 64 hd partitions, then a psum*sbuf multiply into yt
                    fill()
                    ra = lnp.tile([65, 2, 512], mybir.dt.float32r,
                                  tag="ln", name="ra")
                    with nc.allow_low_precision("1/d broadcast via fp32r matmul"):
                        for e in (0, 1):
                            nc.vector.reciprocal(
                                ra[HD:HD + 1, e, 0:n], u2[e][HD:HD + 1, 0:n]
                            )
                    rbs = rcp.tile([64, 2, 512], F16, tag="rc", name="rbs")
                    for e in (0, 1):
                        rb = mmp.tile([128, 512], F32, tag="mm", name="rbps")
                        nc.tensor.matmul(
                            rb[0:64, 0:n],
                            ones64[HD:HD + 1, 0:64],
                            ra[HD:HD + 1, e, 0:n],
                            start=True, stop=True,
                        )
                        nc.vector.tensor_copy(rbs[0:64, e, 0:n], rb[0:64, 0:n])
                        nc.vector.tensor_mul(
                            yt_sb[p][e * 64:(e + 1) * 64, q0:q0 + n],
                            u2[e][0:64, 0:n],
                            rbs[0:64, e, 0:n],
                        )

                # drain remaining fillers before the next chunk's attention
                while fillers:
                    fill()

            # final output projection tiles
            for t in PROJ_TILES[3]:
                for half in (0, 1):
                    emit_proj(t, half)

    _split_excess_waits(nc)
    _BUILD_CACHE["nc"] = nc
    return nc


def _pv(nc, u2, v6_sb, p, n, item, nitems):
    idx, t, kw, qlo, et = item
    for e in (0, 1):
        nc.tensor.matmul(
            u2[e][0:66, qlo:n],
            v6_sb[t][0:kw, 2 * p + e, :],
            et[0:kw, e, qlo:n],
            start=(idx == 0), stop=(idx == nitems - 1),
        )


def _prep_inputs(x, W_attn, W_proj):
    x = np.asarray(x, np.float32)
    W_attn = np.asarray(W_attn, np.float32)
    W_proj = np.asarray(W_proj, np.float32)
    qmr, kmr = _mask_rows()
    xT_by_batch = []
    for b in range(B):
        xt = np.zeros((C, TPP), np.float16)
        xt[:, :T] = x[b][PERM, :].T
        xT_by_batch.append(xt)
    in_maps = []
    for core in range(NCORES):
        b, g = core // 2, core % 2
        qb, kb, vb = [], [], []
        for h in range(NHG):
            gh = g * NHG + h
            qb.append(W_attn[:, gh * HD:(gh + 1) * HD])
            kb.append(W_attn[:, C + gh * HD:C + (gh + 1) * HD])
            vb.append(W_attn[:, 2 * C + gh * HD:2 * C + (gh + 1) * HD])
        wa_core = np.concatenate(qb + kb + vb, axis=1).astype(np.float16)
        wp_core = np.ascontiguousarray(
            W_proj[g * NHG * HD:(g + 1) * NHG * HD, :]
        ).astype(np.float16)
        in_maps.append({
            "xT": xT_by_batch[b],
            "wa": np.ascontiguousarray(wa_core),
            "wp": wp_core,
            "qm": qmr,
            "km": kmr,
        })
    return in_maps


def _run(inputs, trace=False, trace_cores=None):
    nc = _build()
    in_maps = _prep_inputs(inputs["x"], inputs["W_attn"], inputs["W_proj"])
    res = run_bass_kernel_spmd(
        nc, in_maps, list(range(NCORES)), trace=trace, trace_cores=trace_cores
    )
    outs = [np.asarray(r["out"]).astype(np.float32) for r in res.results]
    full = np.empty((B, T, C), np.float32)
    for b in range(B):
        comb = outs[2 * b] + outs[2 * b + 1]
        full[b][PERM, :] = comb
    return full, res


def kernel(**inputs) -> np.ndarray:
    out, _ = _run(inputs)
    return out
